# revision 11
# baseline (speedup 1.0000x reference)
"""BertAttention (B=4, S=2048, H=1024, nH=16) on 8 trn2 NeuronCores.

Sharding: data-parallel over batch (4) x tensor-parallel over heads (2x8),
so core c handles batch c//2 and heads (c%2)*8..(c%2)*8+8. Each core:
  - projects Q,K (transposed layout [d, s]) and V ([s, d], ones-augmented)
  - computes transposed attention scores S^T = K^T(d,k-chunk)^T-matmul per
    head: PSUM [k,q]; ScalarE Exp (mask as per-partition bias) -> expT bf16;
    VectorE copy (+mask) -> f32 scores staged to HBM (transposed; host
    untransposes)
  - context^T + softmax denominators in one matmul chain (V augmented with a
    ones column); normalization fused into the PSUM->SBUF move
  - output dense (partial over local 512 input dims) -> ReduceScatter over
    core pairs -> residual + LayerNorm on half the rows.
"""

import numpy as np
import ml_dtypes

import concourse.bass as bass
import concourse.mybir as mybir
import concourse.tile as tile
from concourse.bass_utils import run_bass_kernel_spmd
from concourse.vector_clock import ScopedClock

f32 = mybir.dt.float32
f32r = mybir.dt.float32r
bf16 = mybir.dt.bfloat16
AF = mybir.ActivationFunctionType
ALU = mybir.AluOpType

B, S, H, NH, DH = 4, 2048, 1024, 16, 64
P = 128
N_CORES = 8
HEADS_PER_CORE = NH // 2  # 8
D_LOCAL = HEADS_PER_CORE * DH  # 512
SCALE = 1.0 / 8.0  # 1/sqrt(DH)
LN_EPS = 1e-5
S_HALF = S // 2

QB = 1024  # q-block size in attention phase
KC = S // P  # 16 k-chunks per head
SC = 512  # s-chunk for projections


class SplitDrainTileContext(tile.TileContext):
    """Walrus in this env rejects >1 sync wait per instruction; the Tile
    kernel-tail drain collects several. Split them across SP nops."""

    def _drain_and_barrier(self, tick_clock, wait_clock):
        nop0 = self.nc.sync.nop(nofuse=True, hint="drain_wait_collect")
        wait_clock.add_sem_waits(
            nop0.ins, ScopedClock({None: tick_clock.global_clock})
        )
        si = nop0.ins.sync_info
        waits = list(si.on_wait) if si and si.on_wait else []
        if len(waits) > 1:
            nop0.ins.sync_info = mybir.SyncInfo(
                on_wait=waits[:1], on_update=list(si.on_update or [])
            )
            for i in range(1, len(waits)):
                nop = self.nc.sync.nop(nofuse=True, hint="drain_wait_split")
                nop.ins.sync_info = mybir.SyncInfo(
                    on_wait=[waits[i]], on_update=[]
                )
        self.nc.sync.drain()
        self.nc.all_engine_barrier()
        assert self.sems is not None
        popped = self.nc._tile_sem_poison_stack.pop()
        assert popped is self._sem_poison
        self.nc.clear_and_free_semaphores(list(self.sems.allocated().values()))
        self.nc.all_engine_barrier()


def _r(ap):
    return ap.bitcast(f32r)


def _split_multi_waits(nc):
    """Walrus in this env accepts at most one sync wait per instruction.
    Hoist extra waits onto same-engine NOPs placed just before."""
    n_split = 0
    for fn in nc.m.functions:
        for bb in fn.blocks:
            new = []
            changed = False
            for inst in bb.instructions:
                si = inst.sync_info
                waits = list(si.on_wait) if si and si.on_wait else []
                if len(waits) > 1:
                    changed = True
                    n_split += 1
                    for w in waits[:-1]:
                        nop = mybir.InstNoOp(
                            name=nc.get_next_instruction_name(), ins=[], outs=[]
                        )
                        nop.engine = inst.engine
                        nop.sync_info = mybir.SyncInfo(on_wait=[w], on_update=[])
                        new.append(nop)
                    inst.sync_info = mybir.SyncInfo(
                        on_wait=[waits[-1]], on_update=list(si.on_update or [])
                    )
                new.append(inst)
            if changed:
                bb.instructions = new
    return n_split


def build_program() -> bass.Bass:
    nc = bass.Bass("TRN2", num_devices=N_CORES)

    xT = nc.declare_dram_parameter("xT", [H, S], f32, isOutput=False)
    hid_half = nc.declare_dram_parameter("hid_half", [S_HALF, H], f32, isOutput=False)
    WqT = nc.declare_dram_parameter("WqT", [H, D_LOCAL], f32, isOutput=False)
    WkT = nc.declare_dram_parameter("WkT", [H, D_LOCAL], f32, isOutput=False)
    WvT = nc.declare_dram_parameter("WvT", [H, D_LOCAL], f32, isOutput=False)
    WoT = nc.declare_dram_parameter("WoT", [D_LOCAL, H], f32, isOutput=False)
    bq_col = nc.declare_dram_parameter("bq_col", [P, 4], f32, isOutput=False)
    bk_col = nc.declare_dram_parameter("bk_col", [P, 4], f32, isOutput=False)
    bv_rep = nc.declare_dram_parameter("bv_rep", [P, D_LOCAL], f32, isOutput=False)
    bo_rep = nc.declare_dram_parameter("bo_rep", [P, H], f32, isOutput=False)
    gamma_rep = nc.declare_dram_parameter("gamma_rep", [P, H], f32, isOutput=False)
    beta_rep = nc.declare_dram_parameter("beta_rep", [P, H], f32, isOutput=False)
    mask_col = nc.declare_dram_parameter("mask_col", [P, KC], f32, isOutput=False)
    ones64 = nc.declare_dram_parameter("ones64", [1, DH], f32, isOutput=False)
    vones = nc.declare_dram_parameter(
        "vones", [P, KC, HEADS_PER_CORE, 1], bf16, isOutput=False
    )
    eps_col = nc.declare_dram_parameter("eps_col", [P, 1], f32, isOutput=False)

    scoresT = nc.declare_dram_parameter(
        "scoresT", [HEADS_PER_CORE, S, S], f32, isOutput=True
    )
    out_half = nc.declare_dram_parameter("out_half", [S_HALF, H], f32, isOutput=True)

    y_part = nc.dram_tensor("y_part", [S, H], f32)
    y_red = nc.dram_tensor("y_red", [S_HALF, H], f32)

    with SplitDrainTileContext(nc) as tc:
        _build_tile(nc, tc, locals())
    _split_multi_waits(nc)
    return nc


def _build_tile(nc, tc, t):
    xT, WqT, WkT, WvT, WoT = t["xT"], t["WqT"], t["WkT"], t["WvT"], t["WoT"]
    bq_col, bk_col, bv_rep = t["bq_col"], t["bk_col"], t["bv_rep"]
    bo_rep, gamma_rep, beta_rep = t["bo_rep"], t["gamma_rep"], t["beta_rep"]
    mask_col, scoresT, out_half = t["mask_col"], t["scoresT"], t["out_half"]
    hid_half, y_part, y_red = t["hid_half"], t["y_part"], t["y_red"]
    ones64, vones, eps_col = t["ones64"], t["vones"], t["eps_col"]

    from contextlib import ExitStack

    with ExitStack() as ctx:
        # ---- persistent pools (live across phases) ----
        persist = ctx.enter_context(tc.tile_pool(name="persist", bufs=1))
        qT_sb = persist.tile([P, 4, S], f32r)        # [d%128, d//128, s]
        kT_sb = persist.tile([P, 4, S], f32r)
        v_aug = persist.tile([P, KC, HEADS_PER_CORE, DH + 1], bf16)
        ctx_sb = persist.tile([P, 4, S], f32r)      # normalized context^T
        mask_sb = persist.tile([P, KC], f32)
        bq_sb = persist.tile([P, 4], f32)
        bk_sb = persist.tile([P, 4], f32)
        bv_sb = persist.tile([P, D_LOCAL], f32)

        nc.sync.dma_start(out=mask_sb[:], in_=mask_col[:])
        nc.sync.dma_start(out=bq_sb[:], in_=bq_col[:])
        nc.sync.dma_start(out=bk_sb[:], in_=bk_col[:])
        nc.sync.dma_start(out=bv_sb[:], in_=bv_rep[:])
        nc.sync.dma_start(out=v_aug[:, :, :, DH : DH + 1], in_=vones[:])
        ones_sb = persist.tile([1, DH], f32r)
        nc.sync.dma_start(out=ones_sb[:], in_=_r(ones64[:]))

        # ================= Phase A: projections =================
        xT_r = xT.rearrange("(c p) s -> p c s", p=P)       # [128, 8, S]
        wq_r = WqT.rearrange("(c p) d -> p c d", p=P)      # [128, 8, 512]
        wk_r = WkT.rearrange("(c p) d -> p c d", p=P)
        wv_r = WvT.rearrange("(c p) d -> p c d", p=P)

        with tc.tile_pool(name="wpool", bufs=1) as wpool, \
             tc.tile_pool(name="xpool", bufs=2) as xpool, \
             tc.tile_pool(name="proj_psum", bufs=4, space="PSUM") as ppsum:
            wq_sb = wpool.tile([P, 8, D_LOCAL], f32r)
            wk_sb = wpool.tile([P, 8, D_LOCAL], f32r)
            wv_sb = wpool.tile([P, 8, D_LOCAL], f32r)
            nc.sync.dma_start(out=wq_sb[:], in_=_r(wq_r))
            nc.sync.dma_start(out=wk_sb[:], in_=_r(wk_r))
            nc.sync.dma_start(out=wv_sb[:], in_=_r(wv_r))

            for jc in range(S // SC):  # 4 s-chunks of 512
                s0 = jc * SC
                x_sb = xpool.tile([P, 8, SC], f32r, tag="x")
                nc.sync.dma_start(out=x_sb[:], in_=_r(xT_r[:, :, s0 : s0 + SC]))

                # Q and K: out qT[d-tile m, s-chunk]
                for w_sb, b_sb, dst, scale in (
                    (wq_sb, bq_sb, qT_sb, SCALE),
                    (wk_sb, bk_sb, kT_sb, 1.0),
                ):
                    for m in range(4):
                        ps = ppsum.tile([P, SC], f32, tag="pp")
                        for c in range(8):
                            nc.tensor.matmul(
                                ps[:],
                                _r(w_sb[:, c, m * P : (m + 1) * P]),
                                _r(x_sb[:, c, :]),
                                start=(c == 0),
                                stop=(c == 7),
                            )
                        nc.vector.tensor_scalar(
                            out=dst[:, m, s0 : s0 + SC],
                            in0=ps[:],
                            scalar1=scale,
                            scalar2=b_sb[:, m : m + 1],
                            op0=ALU.mult,
                            op1=ALU.add,
                        )

                # V: out v[s-tile, d] augmented
                for m in range(4):
                    st = jc * 4 + m  # global s-tile 0..15
                    ps = ppsum.tile([P, D_LOCAL], f32, tag="pp")
                    for c in range(8):
                        nc.tensor.matmul(
                            ps[:],
                            _r(x_sb[:, c, m * P : (m + 1) * P]),
                            _r(wv_sb[:, c, :]),
                            start=(c == 0),
                            stop=(c == 7),
                        )
                    nc.vector.tensor_tensor(
                        out=v_aug[:, st, :, 0:DH],
                        in0=ps[:].rearrange("p (h d) -> p h d", d=DH),
                        in1=bv_sb[:].rearrange("p (h d) -> p h d", d=DH),
                        op=ALU.add,
                    )

        # ================= Phase B: attention =================
        with tc.tile_pool(name="expp", bufs=1) as expp, \
             tc.tile_pool(name="souts", bufs=4) as souts, \
             tc.tile_pool(name="small", bufs=4) as small, \
             tc.tile_pool(name="sc_psum", bufs=2, space="PSUM") as scps, \
             tc.tile_pool(name="cx_psum", bufs=2, space="PSUM") as cxps:
            for p in range(HEADS_PER_CORE):
                tm, po = p // 2, (p % 2) * DH
                for qb in range(S // QB):
                    q0 = qb * QB
                    expT = expp.tile([P, KC, QB], bf16, tag="expT")
                    for kc in range(KC):
                        ps = scps.tile([P, QB], f32, tag="sc")
                        for qn in range(QB // 512):
                            nc.tensor.matmul(
                                ps[:, qn * 512 : (qn + 1) * 512],
                                _r(kT_sb[po : po + DH, tm, kc * P : (kc + 1) * P]),
                                _r(
                                    qT_sb[
                                        po : po + DH,
                                        tm,
                                        q0 + qn * 512 : q0 + (qn + 1) * 512,
                                    ]
                                ),
                                start=True,
                                stop=True,
                            )
                        nc.scalar.activation(
                            out=expT[:, kc, :],
                            in_=ps[:],
                            func=AF.Exp,
                            bias=mask_sb[:, kc : kc + 1],
                            scale=1.0,
                        )
                        so = souts.tile([P, QB], f32, tag="so")
                        nc.vector.tensor_scalar_add(
                            out=so[:], in0=ps[:], scalar1=mask_sb[:, kc : kc + 1]
                        )
                        nc.sync.dma_start(
                            out=scoresT[p, kc * P : (kc + 1) * P, q0 : q0 + QB],
                            in_=so[:],
                        )
                    # context + softmax denominators
                    for qn in range(QB // 512):
                        cps = cxps.tile([DH + 1, 512], f32, tag="cx")
                        for kc in range(KC):
                            nc.tensor.matmul(
                                cps[:],
                                v_aug[:, kc, p, :],
                                expT[:, kc, qn * 512 : (qn + 1) * 512],
                                start=(kc == 0),
                                stop=(kc == KC - 1),
                            )
                        r_row = small.tile([1, 512], f32r, tag="r1")
                        with nc.allow_low_precision(
                            reason="fp32r rounding of softmax reciprocal"
                        ):
                            nc.vector.reciprocal(
                                out=r_row[:], in_=cps[DH : DH + 1, :]
                            )
                        r_ps = cxps.tile([DH, 512], f32, tag="rp")
                        nc.tensor.matmul(
                            r_ps[:], ones_sb[:], r_row[:], start=True, stop=True
                        )
                        r_rep = small.tile([DH, 512], f32, tag="rr")
                        nc.scalar.activation(
                            out=r_rep[:], in_=r_ps[:], func=AF.Copy
                        )
                        nc.vector.tensor_tensor(
                            out=ctx_sb[
                                po : po + DH, tm, q0 + qn * 512 : q0 + (qn + 1) * 512
                            ],
                            in0=cps[0:DH, :],
                            in1=r_rep[:],
                            op=ALU.mult,
                        )

        # ================= Phase C: dense + RS + LN =================
        wo_r = WoT.rearrange("(c p) o -> p c o", p=P)  # [128, 4, H]
        with tc.tile_pool(name="tailp", bufs=2) as tailp, \
             tc.tile_pool(name="tail1", bufs=1) as tail1, \
             tc.tile_pool(name="tail_small", bufs=4) as tsm, \
             tc.tile_pool(name="y_psum", bufs=4, space="PSUM") as yps:
            wo_sb = tail1.tile([P, 4, H], f32r)
            nc.sync.dma_start(out=wo_sb[:], in_=_r(wo_r))

            for st in range(S // P):  # 16 s-tiles
                y_sb = tailp.tile([P, H], f32, tag="y")
                for n in range(2):
                    ps = yps.tile([P, 512], f32, tag="yp")
                    for c in range(4):
                        nc.tensor.matmul(
                            ps[:],
                            _r(ctx_sb[:, c, st * P : (st + 1) * P]),
                            _r(wo_sb[:, c, n * 512 : (n + 1) * 512]),
                            start=(c == 0),
                            stop=(c == 3),
                        )
                    nc.scalar.activation(
                        out=y_sb[:, n * 512 : (n + 1) * 512], in_=ps[:], func=AF.Copy
                    )
                nc.sync.dma_start(out=y_part[st * P : (st + 1) * P, :], in_=y_sb[:])

            nc.gpsimd.collective_compute(
                "ReduceScatter",
                ALU.add,
                replica_groups=[[0, 1], [2, 3], [4, 5], [6, 7]],
                ins=[y_part[:]],
                outs=[y_red[:]],
            )

            bo_sb = tail1.tile([P, H], f32)
            ga_sb = tail1.tile([P, H], f32)
            be_sb = tail1.tile([P, H], f32)
            eps_sb = tail1.tile([P, 1], f32)
            nc.sync.dma_start(out=eps_sb[:], in_=eps_col[:])
            nc.sync.dma_start(out=bo_sb[:], in_=bo_rep[:])
            nc.sync.dma_start(out=ga_sb[:], in_=gamma_rep[:])
            nc.sync.dma_start(out=be_sb[:], in_=beta_rep[:])

            for st in range(S_HALF // P):  # 8 s-tiles
                r0 = st * P
                x_sb = tailp.tile([P, H], f32, tag="xr")
                h_sb = tailp.tile([P, H], f32, tag="hr")
                nc.sync.dma_start(out=x_sb[:], in_=y_red[r0 : r0 + P, :])
                nc.sync.dma_start(out=h_sb[:], in_=hid_half[r0 : r0 + P, :])
                xx = tailp.tile([P, H], f32, tag="xx")
                nc.vector.tensor_tensor(out=xx[:], in0=x_sb[:], in1=h_sb[:], op=ALU.add)
                nc.vector.tensor_tensor(out=xx[:], in0=xx[:], in1=bo_sb[:], op=ALU.add)

                stats = tsm.tile([P, 2, 6], f32, tag="st")
                for g in range(2):
                    nc.vector.bn_stats(
                        out=stats[:, g, :], in_=xx[:, g * 512 : (g + 1) * 512]
                    )
                mv = tsm.tile([P, 2], f32, tag="mv")
                nc.vector.bn_aggr(out=mv[:], in_=stats[:])
                sd = tsm.tile([P, 1], f32, tag="sd")
                nc.scalar.activation(
                    out=sd[:], in_=mv[:, 1:2], func=AF.Sqrt, bias=eps_sb[:], scale=1.0
                )
                rstd = tsm.tile([P, 1], f32, tag="rs")
                nc.vector.reciprocal(out=rstd[:], in_=sd[:])

                xn = tailp.tile([P, H], f32, tag="xn")
                nc.vector.tensor_scalar(
                    out=xn[:],
                    in0=xx[:],
                    scalar1=mv[:, 0:1],
                    scalar2=rstd[:],
                    op0=ALU.subtract,
                    op1=ALU.mult,
                )
                o_sb = tailp.tile([P, H], f32, tag="o")
                nc.vector.tensor_tensor(out=o_sb[:], in0=xn[:], in1=ga_sb[:], op=ALU.mult)
                nc.vector.tensor_tensor(out=o_sb[:], in0=o_sb[:], in1=be_sb[:], op=ALU.add)
                nc.sync.dma_start(out=out_half[r0 : r0 + P, :], in_=o_sb[:])




# ---------------------------------------------------------------------------
# Cached PJRT runner: mirrors concourse.bass2jax.run_bass_via_pjrt but keeps
# the jitted executable + device-resident inputs so repeated calls (and
# timing) skip retrace/recompile/re-upload.
# ---------------------------------------------------------------------------
_RUNNER = None


class _Runner:
    def __init__(self, nc):
        import jax
        from jax.experimental.shard_map import shard_map
        from jax.sharding import Mesh, PartitionSpec
        from concourse import bass2jax
        from concourse import mybir as _mybir

        bass2jax.install_neuronx_cc_hook()
        self.jax = jax
        self.nc = nc
        partition_name = (
            nc.partition_id_tensor.name if nc.partition_id_tensor else None
        )
        in_names, out_names, out_avals = [], [], []
        for alloc in nc.m.functions[0].allocations:
            if not isinstance(alloc, _mybir.MemoryLocationSet):
                continue
            name = alloc.memorylocations[0].name
            if alloc.kind == "ExternalInput":
                if name != partition_name:
                    in_names.append(name)
            elif alloc.kind == "ExternalOutput":
                out_names.append(name)
                out_avals.append(
                    jax.core.ShapedArray(
                        tuple(alloc.tensor_shape), _mybir.dt.np(alloc.dtype)
                    )
                )
        self.in_names = list(in_names)
        self.out_names = out_names
        self.out_avals = out_avals
        n_params = len(in_names)
        n_outs = len(out_names)
        all_in_names = in_names + out_names
        if partition_name is not None:
            all_in_names.append(partition_name)

        def _body(*args):
            operands = list(args)
            if partition_name is not None:
                operands.append(bass2jax.partition_id_tensor())
            return tuple(
                bass2jax._bass_exec_p.bind(
                    *operands,
                    out_avals=tuple(out_avals),
                    in_names=tuple(all_in_names),
                    out_names=tuple(out_names),
                    lowering_input_output_aliases=(),
                    sim_require_finite=True,
                    sim_require_nnan=True,
                    nc=nc,
                )
            )

        devices = jax.devices()[:N_CORES]
        self.mesh = Mesh(__import__("numpy").asarray(devices), ("core",))
        in_specs = (PartitionSpec("core"),) * (n_params + n_outs)
        out_specs = (PartitionSpec("core"),) * n_outs
        self.sharded = jax.jit(
            shard_map(
                _body,
                mesh=self.mesh,
                in_specs=in_specs,
                out_specs=out_specs,
                check_rep=False,
            ),
            donate_argnums=tuple(range(n_params, n_params + n_outs)),
            keep_unused=True,
        )

    def _zeros(self):
        import jax.numpy as jnp
        from jax.sharding import NamedSharding, PartitionSpec

        outs = []
        for av in self.out_avals:
            shp = (N_CORES * av.shape[0],) + tuple(av.shape[1:])
            outs.append(
                self.jax.device_put(
                    jnp.zeros(shp, av.dtype),
                    NamedSharding(self.mesh, PartitionSpec("core")),
                )
            )
        return outs

    def prepare_inputs(self, in_maps):
        import jax
        from jax.sharding import NamedSharding, PartitionSpec

        sh = NamedSharding(self.mesh, PartitionSpec("core"))
        concat = [
            np.concatenate([np.asarray(m[name]) for m in in_maps], axis=0)
            for name in self.in_names
        ]
        return [jax.device_put(a, sh) for a in concat]

    def run(self, dev_inputs):
        outs = self.sharded(*dev_inputs, *self._zeros())
        self.jax.block_until_ready(outs)
        return outs

    def run_host(self, in_maps):
        dev_inputs = self.prepare_inputs(in_maps)
        outs = self.run(dev_inputs)
        res = []
        for c in range(N_CORES):
            d = {}
            for i, name in enumerate(self.out_names):
                d[name] = np.asarray(outs[i]).reshape(
                    N_CORES, *self.out_avals[i].shape
                )[c]
            res.append(d)
        return res


def _get_runner():
    global _RUNNER
    if _RUNNER is None:
        _RUNNER = _Runner(_get_nc())
    return _RUNNER


_NC_CACHE = None


def _get_nc():
    global _NC_CACHE
    if _NC_CACHE is None:
        _NC_CACHE = build_program()
    return _NC_CACHE


def _shard_inputs(hidden_states, attn_mask, Wq, bq, Wk, bk, Wv, bv, Wo, bo):
    c = np.ascontiguousarray
    WqT_f, WkT_f, WvT_f, WoT_f = Wq.T, Wk.T, Wv.T, Wo.T
    in_maps = []
    for core in range(N_CORES):
        b, hh = core // 2, core % 2
        r0, r1 = hh * D_LOCAL, (hh + 1) * D_LOCAL
        half = hh  # head-half also picks the output row-half
        in_maps.append(
            {
                "xT": c(hidden_states[b].T),
                "hid_half": c(hidden_states[b, half * S_HALF : (half + 1) * S_HALF]),
                "WqT": c(WqT_f[:, r0:r1]),
                "WkT": c(WkT_f[:, r0:r1]),
                "WvT": c(WvT_f[:, r0:r1]),
                "WoT": c(WoT_f[r0:r1, :]),
                "bq_col": c((bq[r0:r1] * SCALE).reshape(4, P).T),
                "bk_col": c(bk[r0:r1].reshape(4, P).T),
                "bv_rep": c(np.broadcast_to(bv[r0:r1], (P, D_LOCAL))),
                "bo_rep": c(np.broadcast_to(bo, (P, H))),
                "gamma_rep": None,  # filled by caller
                "beta_rep": None,
                "mask_col": c((attn_mask[b] * -10000.0).reshape(KC, P).T),
                "ones64": np.ones((1, DH), np.float32),
                "vones": np.ones((P, KC, HEADS_PER_CORE, 1), ml_dtypes.bfloat16),
                "eps_col": np.full((P, 1), LN_EPS, np.float32),
            }
        )
    return in_maps


def kernel(
    hidden_states,
    attn_mask,
    Wq,
    bq,
    Wk,
    bk,
    Wv,
    bv,
    Wo,
    bo,
    gamma,
    beta,
):
    hidden_states = np.asarray(hidden_states, dtype=np.float32)
    attn_mask = np.asarray(attn_mask, dtype=np.float32)
    args = [np.asarray(a, dtype=np.float32) for a in (Wq, bq, Wk, bk, Wv, bv, Wo, bo)]
    gamma = np.asarray(gamma, dtype=np.float32)
    beta = np.asarray(beta, dtype=np.float32)

    nc = _get_nc()
    in_maps = _shard_inputs(hidden_states, attn_mask, *args)
    c = np.ascontiguousarray
    ga = c(np.broadcast_to(gamma, (P, H)))
    be = c(np.broadcast_to(beta, (P, H)))
    for m in in_maps:
        m["gamma_rep"] = ga
        m["beta_rep"] = be

    results = _get_runner().run_host(in_maps)

    output = np.empty((B, S, H), dtype=np.float32)
    attn_score = np.empty((B, NH, S, S), dtype=np.float32)
    for core in range(N_CORES):
        b, hh = core // 2, core % 2
        r = results[core]
        output[b, hh * S_HALF : (hh + 1) * S_HALF] = r["out_half"]
        sT = r["scoresT"]  # [8, S(k), S(q)]
        for p in range(HEADS_PER_CORE):
            attn_score[b, hh * HEADS_PER_CORE + p] = sT[p].T
    return output, attn_score


# revision 13
# speedup vs baseline: 2.2540x; 2.2540x over previous
"""BertAttention (B=4, S=2048, H=1024, nH=16) on 8 trn2 NeuronCores.

Sharding: data-parallel over batch (4) x tensor-parallel over heads (2x8),
so core c handles batch c//2 and heads (c%2)*8..(c%2)*8+8. Each core:
  - projects Q,K (transposed layout [d, s]) and V ([s, d], ones-augmented)
  - computes transposed attention scores S^T = K^T(d,k-chunk)^T-matmul per
    head: PSUM [k,q]; ScalarE Exp (mask as per-partition bias) -> expT bf16;
    VectorE copy (+mask) -> f32 scores staged to HBM (transposed; host
    untransposes)
  - context^T + softmax denominators in one matmul chain (V augmented with a
    ones column); normalization fused into the PSUM->SBUF move
  - output dense (partial over local 512 input dims) -> ReduceScatter over
    core pairs -> residual + LayerNorm on half the rows.
"""

import numpy as np
import ml_dtypes

import concourse.bass as bass
import concourse.mybir as mybir
import concourse.tile as tile
from concourse.bass_utils import run_bass_kernel_spmd
from concourse.vector_clock import ScopedClock

f32 = mybir.dt.float32
f32r = mybir.dt.float32r
bf16 = mybir.dt.bfloat16
AF = mybir.ActivationFunctionType
ALU = mybir.AluOpType

B, S, H, NH, DH = 4, 2048, 1024, 16, 64
P = 128
N_CORES = 8
HEADS_PER_CORE = NH // 2  # 8
D_LOCAL = HEADS_PER_CORE * DH  # 512
SCALE = 1.0 / 8.0  # 1/sqrt(DH)
LN_EPS = 1e-5
S_HALF = S // 2

QB = 1024  # q-block size in attention phase
KC = S // P  # 16 k-chunks per head
SC = 512  # s-chunk for projections


class SplitDrainTileContext(tile.TileContext):
    """Walrus in this env rejects >1 sync wait per instruction; the Tile
    kernel-tail drain collects several. Split them across SP nops."""

    def _drain_and_barrier(self, tick_clock, wait_clock):
        nop0 = self.nc.sync.nop(nofuse=True, hint="drain_wait_collect")
        wait_clock.add_sem_waits(
            nop0.ins, ScopedClock({None: tick_clock.global_clock})
        )
        si = nop0.ins.sync_info
        waits = list(si.on_wait) if si and si.on_wait else []
        if len(waits) > 1:
            nop0.ins.sync_info = mybir.SyncInfo(
                on_wait=waits[:1], on_update=list(si.on_update or [])
            )
            for i in range(1, len(waits)):
                nop = self.nc.sync.nop(nofuse=True, hint="drain_wait_split")
                nop.ins.sync_info = mybir.SyncInfo(
                    on_wait=[waits[i]], on_update=[]
                )
        self.nc.sync.drain()
        self.nc.all_engine_barrier()
        assert self.sems is not None
        popped = self.nc._tile_sem_poison_stack.pop()
        assert popped is self._sem_poison
        self.nc.clear_and_free_semaphores(list(self.sems.allocated().values()))
        self.nc.all_engine_barrier()


def _r(ap):
    return ap.bitcast(f32r)


def _split_multi_waits(nc):
    """Walrus in this env accepts at most one sync wait per instruction.
    Hoist extra waits onto same-engine NOPs placed just before."""
    n_split = 0
    for fn in nc.m.functions:
        for bb in fn.blocks:
            new = []
            changed = False
            for inst in bb.instructions:
                si = inst.sync_info
                waits = list(si.on_wait) if si and si.on_wait else []
                if len(waits) > 1:
                    changed = True
                    n_split += 1
                    for w in waits[:-1]:
                        nop = mybir.InstNoOp(
                            name=nc.get_next_instruction_name(), ins=[], outs=[]
                        )
                        nop.engine = inst.engine
                        nop.sync_info = mybir.SyncInfo(on_wait=[w], on_update=[])
                        new.append(nop)
                    inst.sync_info = mybir.SyncInfo(
                        on_wait=[waits[-1]], on_update=list(si.on_update or [])
                    )
                new.append(inst)
            if changed:
                bb.instructions = new
    return n_split


def build_program() -> bass.Bass:
    nc = bass.Bass("TRN2", num_devices=N_CORES)

    xT = nc.declare_dram_parameter("xT", [H, S], f32, isOutput=False)
    hid_half = nc.declare_dram_parameter("hid_half", [S_HALF, H], f32, isOutput=False)
    WqT = nc.declare_dram_parameter("WqT", [H, D_LOCAL], f32, isOutput=False)
    WkT = nc.declare_dram_parameter("WkT", [H, D_LOCAL], f32, isOutput=False)
    WvT = nc.declare_dram_parameter("WvT", [H, D_LOCAL], f32, isOutput=False)
    WoT = nc.declare_dram_parameter("WoT", [D_LOCAL, H], f32, isOutput=False)
    bq_col = nc.declare_dram_parameter("bq_col", [P, 4], f32, isOutput=False)
    bk_col = nc.declare_dram_parameter("bk_col", [P, 4], f32, isOutput=False)
    bv_rep = nc.declare_dram_parameter("bv_rep", [P, D_LOCAL], f32, isOutput=False)
    bo_rep = nc.declare_dram_parameter("bo_rep", [P, H], f32, isOutput=False)
    gamma_rep = nc.declare_dram_parameter("gamma_rep", [P, H], f32, isOutput=False)
    beta_rep = nc.declare_dram_parameter("beta_rep", [P, H], f32, isOutput=False)
    mask_col = nc.declare_dram_parameter("mask_col", [P, KC], f32, isOutput=False)
    ones64 = nc.declare_dram_parameter("ones64", [1, DH], f32, isOutput=False)
    vones = nc.declare_dram_parameter(
        "vones", [P, KC, HEADS_PER_CORE, 1], bf16, isOutput=False
    )
    eps_col = nc.declare_dram_parameter("eps_col", [P, 1], f32, isOutput=False)

    scoresT = nc.declare_dram_parameter(
        "scoresT", [HEADS_PER_CORE, S, S], f32, isOutput=True
    )
    out_half = nc.declare_dram_parameter("out_half", [S_HALF, H], f32, isOutput=True)

    y_part = nc.dram_tensor("y_part", [S, H], f32)
    y_red = nc.dram_tensor("y_red", [S_HALF, H], f32)

    with SplitDrainTileContext(nc) as tc:
        _build_tile(nc, tc, locals())
    _split_multi_waits(nc)
    return nc


def _build_tile(nc, tc, t):
    xT, WqT, WkT, WvT, WoT = t["xT"], t["WqT"], t["WkT"], t["WvT"], t["WoT"]
    bq_col, bk_col, bv_rep = t["bq_col"], t["bk_col"], t["bv_rep"]
    bo_rep, gamma_rep, beta_rep = t["bo_rep"], t["gamma_rep"], t["beta_rep"]
    mask_col, scoresT, out_half = t["mask_col"], t["scoresT"], t["out_half"]
    hid_half, y_part, y_red = t["hid_half"], t["y_part"], t["y_red"]
    ones64, vones, eps_col = t["ones64"], t["vones"], t["eps_col"]

    from contextlib import ExitStack

    with ExitStack() as ctx:
        # ---- persistent pools (live across phases) ----
        persist = ctx.enter_context(tc.tile_pool(name="persist", bufs=1))
        qT_sb = persist.tile([P, 4, S], f32r)        # [d%128, d//128, s]
        kT_sb = persist.tile([P, 4, S], f32r)
        v_aug = persist.tile([P, KC, HEADS_PER_CORE, DH + 1], bf16)
        ctx_sb = persist.tile([P, 4, S], f32r)      # normalized context^T
        mask_sb = persist.tile([P, KC], f32)
        bq_sb = persist.tile([P, 4], f32)
        bk_sb = persist.tile([P, 4], f32)
        bv_sb = persist.tile([P, D_LOCAL], f32)

        nc.sync.dma_start(out=mask_sb[:], in_=mask_col[:])
        nc.sync.dma_start(out=bq_sb[:], in_=bq_col[:])
        nc.sync.dma_start(out=bk_sb[:], in_=bk_col[:])
        nc.sync.dma_start(out=bv_sb[:], in_=bv_rep[:])
        nc.sync.dma_start(out=v_aug[:, :, :, DH : DH + 1], in_=vones[:])
        ones_sb = persist.tile([1, DH], f32r)
        nc.sync.dma_start(out=ones_sb[:], in_=_r(ones64[:]))

        # ================= Phase A: projections =================
        xT_r = xT.rearrange("(c p) s -> p c s", p=P)       # [128, 8, S]
        wq_r = WqT.rearrange("(c p) d -> p c d", p=P)      # [128, 8, 512]
        wk_r = WkT.rearrange("(c p) d -> p c d", p=P)
        wv_r = WvT.rearrange("(c p) d -> p c d", p=P)

        with tc.tile_pool(name="wpool", bufs=1) as wpool, \
             tc.tile_pool(name="xpool", bufs=2) as xpool, \
             tc.tile_pool(name="proj_psum", bufs=4, space="PSUM") as ppsum:
            wq_sb = wpool.tile([P, 8, D_LOCAL], f32r)
            wk_sb = wpool.tile([P, 8, D_LOCAL], f32r)
            wv_sb = wpool.tile([P, 8, D_LOCAL], f32r)
            nc.sync.dma_start(out=wq_sb[:], in_=_r(wq_r))
            nc.sync.dma_start(out=wk_sb[:], in_=_r(wk_r))
            nc.sync.dma_start(out=wv_sb[:], in_=_r(wv_r))

            for jc in range(S // SC):  # 4 s-chunks of 512
                s0 = jc * SC
                x_sb = xpool.tile([P, 8, SC], f32r, tag="x")
                nc.sync.dma_start(out=x_sb[:], in_=_r(xT_r[:, :, s0 : s0 + SC]))

                # Q and K: out qT[d-tile m, s-chunk]
                for w_sb, b_sb, dst, scale in (
                    (wq_sb, bq_sb, qT_sb, SCALE),
                    (wk_sb, bk_sb, kT_sb, 1.0),
                ):
                    for m in range(4):
                        ps = ppsum.tile([P, SC], f32, tag="pp")
                        for c in range(8):
                            nc.tensor.matmul(
                                ps[:],
                                _r(w_sb[:, c, m * P : (m + 1) * P]),
                                _r(x_sb[:, c, :]),
                                start=(c == 0),
                                stop=(c == 7),
                            )
                        nc.vector.tensor_scalar(
                            out=dst[:, m, s0 : s0 + SC],
                            in0=ps[:],
                            scalar1=scale,
                            scalar2=b_sb[:, m : m + 1],
                            op0=ALU.mult,
                            op1=ALU.add,
                        )

                # V: out v[s-tile, d] augmented
                for m in range(4):
                    st = jc * 4 + m  # global s-tile 0..15
                    ps = ppsum.tile([P, D_LOCAL], f32, tag="pp")
                    for c in range(8):
                        nc.tensor.matmul(
                            ps[:],
                            _r(x_sb[:, c, m * P : (m + 1) * P]),
                            _r(wv_sb[:, c, :]),
                            start=(c == 0),
                            stop=(c == 7),
                        )
                    nc.vector.tensor_tensor(
                        out=v_aug[:, st, :, 0:DH],
                        in0=ps[:].rearrange("p (h d) -> p h d", d=DH),
                        in1=bv_sb[:].rearrange("p (h d) -> p h d", d=DH),
                        op=ALU.add,
                    )

        # ================= Phase B: attention =================
        with tc.tile_pool(name="expp", bufs=1) as expp, \
             tc.tile_pool(name="souts", bufs=4) as souts, \
             tc.tile_pool(name="small", bufs=4) as small, \
             tc.tile_pool(name="sc_psum", bufs=2, space="PSUM") as scps, \
             tc.tile_pool(name="cx_psum", bufs=2, space="PSUM") as cxps:
            for p in range(HEADS_PER_CORE):
                tm, po = p // 2, (p % 2) * DH
                for qb in range(S // QB):
                    q0 = qb * QB
                    expT = expp.tile([P, KC, QB], bf16, tag="expT")
                    for kc in range(KC):
                        ps = scps.tile([P, QB], f32, tag="sc")
                        for qn in range(QB // 512):
                            nc.tensor.matmul(
                                ps[:, qn * 512 : (qn + 1) * 512],
                                _r(kT_sb[po : po + DH, tm, kc * P : (kc + 1) * P]),
                                _r(
                                    qT_sb[
                                        po : po + DH,
                                        tm,
                                        q0 + qn * 512 : q0 + (qn + 1) * 512,
                                    ]
                                ),
                                start=True,
                                stop=True,
                            )
                        nc.scalar.activation(
                            out=expT[:, kc, :],
                            in_=ps[:],
                            func=AF.Exp,
                            bias=mask_sb[:, kc : kc + 1],
                            scale=1.0,
                        )
                        so = souts.tile([P, QB], f32, tag="so")
                        nc.vector.tensor_scalar_add(
                            out=so[:], in0=ps[:], scalar1=mask_sb[:, kc : kc + 1]
                        )
                        nc.sync.dma_start(
                            out=scoresT[p, kc * P : (kc + 1) * P, q0 : q0 + QB],
                            in_=so[:],
                        )
                    # context + softmax denominators
                    for qn in range(QB // 512):
                        cps = cxps.tile([DH + 1, 512], f32, tag="cx")
                        for kc in range(KC):
                            nc.tensor.matmul(
                                cps[:],
                                v_aug[:, kc, p, :],
                                expT[:, kc, qn * 512 : (qn + 1) * 512],
                                start=(kc == 0),
                                stop=(kc == KC - 1),
                            )
                        r_row = small.tile([1, 512], f32r, tag="r1")
                        with nc.allow_low_precision(
                            reason="fp32r rounding of softmax reciprocal"
                        ):
                            nc.vector.reciprocal(
                                out=r_row[:], in_=cps[DH : DH + 1, :]
                            )
                        r_ps = cxps.tile([DH, 512], f32, tag="rp")
                        nc.tensor.matmul(
                            r_ps[:], ones_sb[:], r_row[:], start=True, stop=True
                        )
                        r_rep = small.tile([DH, 512], f32, tag="rr")
                        nc.scalar.activation(
                            out=r_rep[:], in_=r_ps[:], func=AF.Copy
                        )
                        nc.vector.tensor_tensor(
                            out=ctx_sb[
                                po : po + DH, tm, q0 + qn * 512 : q0 + (qn + 1) * 512
                            ],
                            in0=cps[0:DH, :],
                            in1=r_rep[:],
                            op=ALU.mult,
                        )

        # ================= Phase C: dense + RS + LN =================
        wo_r = WoT.rearrange("(c p) o -> p c o", p=P)  # [128, 4, H]
        with tc.tile_pool(name="tailp", bufs=2) as tailp, \
             tc.tile_pool(name="tail1", bufs=1) as tail1, \
             tc.tile_pool(name="tail_small", bufs=4) as tsm, \
             tc.tile_pool(name="y_psum", bufs=4, space="PSUM") as yps:
            wo_sb = tail1.tile([P, 4, H], f32r)
            nc.sync.dma_start(out=wo_sb[:], in_=_r(wo_r))

            for st in range(S // P):  # 16 s-tiles
                y_sb = tailp.tile([P, H], f32, tag="y")
                for n in range(2):
                    ps = yps.tile([P, 512], f32, tag="yp")
                    for c in range(4):
                        nc.tensor.matmul(
                            ps[:],
                            _r(ctx_sb[:, c, st * P : (st + 1) * P]),
                            _r(wo_sb[:, c, n * 512 : (n + 1) * 512]),
                            start=(c == 0),
                            stop=(c == 3),
                        )
                    nc.scalar.activation(
                        out=y_sb[:, n * 512 : (n + 1) * 512], in_=ps[:], func=AF.Copy
                    )
                nc.sync.dma_start(out=y_part[st * P : (st + 1) * P, :], in_=y_sb[:])

            nc.gpsimd.collective_compute(
                "ReduceScatter",
                ALU.add,
                replica_groups=[[0, 1], [2, 3], [4, 5], [6, 7]],
                ins=[y_part[:]],
                outs=[y_red[:]],
            )

            bo_sb = tail1.tile([P, H], f32)
            ga_sb = tail1.tile([P, H], f32)
            be_sb = tail1.tile([P, H], f32)
            eps_sb = tail1.tile([P, 1], f32)
            nc.sync.dma_start(out=eps_sb[:], in_=eps_col[:])
            nc.sync.dma_start(out=bo_sb[:], in_=bo_rep[:])
            nc.sync.dma_start(out=ga_sb[:], in_=gamma_rep[:])
            nc.sync.dma_start(out=be_sb[:], in_=beta_rep[:])

            for st in range(S_HALF // P):  # 8 s-tiles
                r0 = st * P
                x_sb = tailp.tile([P, H], f32, tag="xr")
                h_sb = tailp.tile([P, H], f32, tag="hr")
                nc.sync.dma_start(out=x_sb[:], in_=y_red[r0 : r0 + P, :])
                nc.sync.dma_start(out=h_sb[:], in_=hid_half[r0 : r0 + P, :])
                xx = tailp.tile([P, H], f32, tag="xx")
                nc.vector.tensor_tensor(out=xx[:], in0=x_sb[:], in1=h_sb[:], op=ALU.add)
                nc.vector.tensor_tensor(out=xx[:], in0=xx[:], in1=bo_sb[:], op=ALU.add)

                stats = tsm.tile([P, 2, 6], f32, tag="st")
                for g in range(2):
                    nc.vector.bn_stats(
                        out=stats[:, g, :], in_=xx[:, g * 512 : (g + 1) * 512]
                    )
                mv = tsm.tile([P, 2], f32, tag="mv")
                nc.vector.bn_aggr(out=mv[:], in_=stats[:])
                sd = tsm.tile([P, 1], f32, tag="sd")
                nc.scalar.activation(
                    out=sd[:], in_=mv[:, 1:2], func=AF.Sqrt, bias=eps_sb[:], scale=1.0
                )
                rstd = tsm.tile([P, 1], f32, tag="rs")
                nc.vector.reciprocal(out=rstd[:], in_=sd[:])

                xn = tailp.tile([P, H], f32, tag="xn")
                nc.vector.tensor_scalar(
                    out=xn[:],
                    in0=xx[:],
                    scalar1=mv[:, 0:1],
                    scalar2=rstd[:],
                    op0=ALU.subtract,
                    op1=ALU.mult,
                )
                o_sb = tailp.tile([P, H], f32, tag="o")
                nc.vector.tensor_tensor(out=o_sb[:], in0=xn[:], in1=ga_sb[:], op=ALU.mult)
                nc.vector.tensor_tensor(out=o_sb[:], in0=o_sb[:], in1=be_sb[:], op=ALU.add)
                nc.sync.dma_start(out=out_half[r0 : r0 + P, :], in_=o_sb[:])




# ---------------------------------------------------------------------------
# Cached PJRT runner: mirrors concourse.bass2jax.run_bass_via_pjrt but keeps
# the jitted executable + device-resident inputs so repeated calls (and
# timing) skip retrace/recompile/re-upload.
# ---------------------------------------------------------------------------
_RUNNER = None


class _Runner:
    def __init__(self, nc):
        import jax
        from jax.experimental.shard_map import shard_map
        from jax.sharding import Mesh, PartitionSpec
        from concourse import bass2jax
        from concourse import mybir as _mybir

        bass2jax.install_neuronx_cc_hook()
        self.jax = jax
        self.nc = nc
        partition_name = (
            nc.partition_id_tensor.name if nc.partition_id_tensor else None
        )
        in_names, out_names, out_avals = [], [], []
        for alloc in nc.m.functions[0].allocations:
            if not isinstance(alloc, _mybir.MemoryLocationSet):
                continue
            name = alloc.memorylocations[0].name
            if alloc.kind == "ExternalInput":
                if name != partition_name:
                    in_names.append(name)
            elif alloc.kind == "ExternalOutput":
                out_names.append(name)
                out_avals.append(
                    jax.core.ShapedArray(
                        tuple(alloc.tensor_shape), _mybir.dt.np(alloc.dtype)
                    )
                )
        self.in_names = list(in_names)
        self.out_names = out_names
        self.out_avals = out_avals
        n_params = len(in_names)
        n_outs = len(out_names)
        all_in_names = in_names + out_names
        if partition_name is not None:
            all_in_names.append(partition_name)

        def _body(*args):
            operands = list(args)
            if partition_name is not None:
                operands.append(bass2jax.partition_id_tensor())
            return tuple(
                bass2jax._bass_exec_p.bind(
                    *operands,
                    out_avals=tuple(out_avals),
                    in_names=tuple(all_in_names),
                    out_names=tuple(out_names),
                    lowering_input_output_aliases=(),
                    sim_require_finite=True,
                    sim_require_nnan=True,
                    nc=nc,
                )
            )

        devices = jax.devices()[:N_CORES]
        self.mesh = Mesh(__import__("numpy").asarray(devices), ("core",))
        in_specs = (PartitionSpec("core"),) * (n_params + n_outs)
        out_specs = (PartitionSpec("core"),) * n_outs
        self.sharded = jax.jit(
            shard_map(
                _body,
                mesh=self.mesh,
                in_specs=in_specs,
                out_specs=out_specs,
                check_rep=False,
            ),
            donate_argnums=tuple(range(n_params, n_params + n_outs)),
            keep_unused=True,
        )
        import jax.numpy as jnp
        from jax.sharding import NamedSharding

        sh = NamedSharding(self.mesh, PartitionSpec("core"))
        shapes = [
            ((N_CORES * av.shape[0],) + tuple(av.shape[1:]), av.dtype)
            for av in out_avals
        ]
        self._zeros_fn = jax.jit(
            lambda: tuple(jnp.zeros(s, d) for s, d in shapes),
            out_shardings=tuple(sh for _ in shapes),
        )

    def prepare_inputs(self, in_maps):
        import jax
        from jax.sharding import NamedSharding, PartitionSpec

        sh = NamedSharding(self.mesh, PartitionSpec("core"))
        concat = [
            np.concatenate([np.asarray(m[name]) for m in in_maps], axis=0)
            for name in self.in_names
        ]
        return [jax.device_put(a, sh) for a in concat]

    def run(self, dev_inputs):
        zeros = self._zeros_fn()
        outs = self.sharded(*dev_inputs, *zeros)
        self.jax.block_until_ready(outs)
        return outs

    def run_host(self, in_maps):
        dev_inputs = self.prepare_inputs(in_maps)
        outs = self.run(dev_inputs)
        res = []
        for c in range(N_CORES):
            d = {}
            for i, name in enumerate(self.out_names):
                d[name] = np.asarray(outs[i]).reshape(
                    N_CORES, *self.out_avals[i].shape
                )[c]
            res.append(d)
        return res


def _get_runner():
    global _RUNNER
    if _RUNNER is None:
        _RUNNER = _Runner(_get_nc())
    return _RUNNER


_NC_CACHE = None


def _get_nc():
    global _NC_CACHE
    if _NC_CACHE is None:
        _NC_CACHE = build_program()
    return _NC_CACHE


def _shard_inputs(hidden_states, attn_mask, Wq, bq, Wk, bk, Wv, bv, Wo, bo):
    c = np.ascontiguousarray
    WqT_f, WkT_f, WvT_f, WoT_f = Wq.T, Wk.T, Wv.T, Wo.T
    in_maps = []
    for core in range(N_CORES):
        b, hh = core // 2, core % 2
        r0, r1 = hh * D_LOCAL, (hh + 1) * D_LOCAL
        half = hh  # head-half also picks the output row-half
        in_maps.append(
            {
                "xT": c(hidden_states[b].T),
                "hid_half": c(hidden_states[b, half * S_HALF : (half + 1) * S_HALF]),
                "WqT": c(WqT_f[:, r0:r1]),
                "WkT": c(WkT_f[:, r0:r1]),
                "WvT": c(WvT_f[:, r0:r1]),
                "WoT": c(WoT_f[r0:r1, :]),
                "bq_col": c((bq[r0:r1] * SCALE).reshape(4, P).T),
                "bk_col": c(bk[r0:r1].reshape(4, P).T),
                "bv_rep": c(np.broadcast_to(bv[r0:r1], (P, D_LOCAL))),
                "bo_rep": c(np.broadcast_to(bo, (P, H))),
                "gamma_rep": None,  # filled by caller
                "beta_rep": None,
                "mask_col": c((attn_mask[b] * -10000.0).reshape(KC, P).T),
                "ones64": np.ones((1, DH), np.float32),
                "vones": np.ones((P, KC, HEADS_PER_CORE, 1), ml_dtypes.bfloat16),
                "eps_col": np.full((P, 1), LN_EPS, np.float32),
            }
        )
    return in_maps


def kernel(
    hidden_states,
    attn_mask,
    Wq,
    bq,
    Wk,
    bk,
    Wv,
    bv,
    Wo,
    bo,
    gamma,
    beta,
):
    hidden_states = np.asarray(hidden_states, dtype=np.float32)
    attn_mask = np.asarray(attn_mask, dtype=np.float32)
    args = [np.asarray(a, dtype=np.float32) for a in (Wq, bq, Wk, bk, Wv, bv, Wo, bo)]
    gamma = np.asarray(gamma, dtype=np.float32)
    beta = np.asarray(beta, dtype=np.float32)

    nc = _get_nc()
    in_maps = _shard_inputs(hidden_states, attn_mask, *args)
    c = np.ascontiguousarray
    ga = c(np.broadcast_to(gamma, (P, H)))
    be = c(np.broadcast_to(beta, (P, H)))
    for m in in_maps:
        m["gamma_rep"] = ga
        m["beta_rep"] = be

    results = _get_runner().run_host(in_maps)

    output = np.empty((B, S, H), dtype=np.float32)
    attn_score = np.empty((B, NH, S, S), dtype=np.float32)
    for core in range(N_CORES):
        b, hh = core // 2, core % 2
        r = results[core]
        output[b, hh * S_HALF : (hh + 1) * S_HALF] = r["out_half"]
        sT = r["scoresT"]  # [8, S(k), S(q)]
        for p in range(HEADS_PER_CORE):
            attn_score[b, hh * HEADS_PER_CORE + p] = sT[p].T
    return output, attn_score
